# revision 6
# baseline (speedup 1.0000x reference)
"""Two-layer GATv2 (DGL-style, eval mode) on 8 Trainium2 NeuronCores.

Edge-parallel by destination range: host sorts edges by dst, splits nodes
into 8 contiguous ranges with ~equal edge counts, and packs each range's
dst nodes into tiles of <=128 edges / <=16 segments. One SPMD program:
P0 dense projections (neighbor-side features stored as a bf16 hi/lo pair so
the expansion matmuls run at bf16 rate with ~fp32 accuracy), P1 layer-1 edge
tiles (indirect-DMA gather of source rows, one-hot PE matmuls for neighbor
expansion and segment aggregation, softmax without max-subtraction, ELU),
P2 layer-2 projections + AllGather, P3 layer-2 edge tiles. Host reassembles
the [N, 64] output.
"""
import numpy as np
import ml_dtypes

import concourse.bass as bass
import concourse.tile as tile
from concourse import bacc, mybir
from concourse.bass_utils import run_bass_kernel_spmd
from concourse.masks import make_identity

F32 = mybir.dt.float32
BF16 = mybir.dt.bfloat16
I32 = mybir.dt.int32
AL = mybir.AluOpType

EPT = 128   # edges per tile
SPT = 16    # segments (dst nodes) per tile
NEG_SLOPE = 0.2


def _prep(src, dst, n_nodes, n_cores=8):
    """Partition + tile the graph. Returns metadata dict."""
    E = src.shape[0]
    src = src.astype(np.int64)
    dst = dst.astype(np.int64)
    order = np.argsort(dst, kind="stable")
    src_s = src[order].astype(np.int32)
    dst_s = dst[order].astype(np.int32)
    deg = np.bincount(dst_s, minlength=n_nodes).astype(np.int64)
    assert deg.max() <= EPT, f"segment larger than a tile: {deg.max()}"
    # node-aligned core boundaries with ~equal edges
    cum = np.cumsum(deg)
    bounds = [0]
    for k in range(1, n_cores):
        t = k * E / n_cores
        bounds.append(int(np.searchsorted(cum, t)))
    bounds.append(n_nodes)
    seg_start = np.concatenate([[0], cum]).astype(np.int64)  # edge offset per node

    cores = []
    for k in range(n_cores):
        v0, v1 = bounds[k], bounds[k + 1]
        tiles = []  # list of (node_lo, node_hi) per tile
        v = v0
        while v < v1:
            ne, ns, vstart = 0, 0, v
            while v < v1 and ns < SPT and ne + deg[v] <= EPT:
                ne += deg[v]; ns += 1; v += 1
            tiles.append((vstart, v))
        cores.append((v0, v1, tiles))
    T = max(len(c[2]) for c in cores)
    T = ((T + 7) // 8) * 8  # multiple of 8 for group finalize

    meta = {
        "T": T, "n_cores": n_cores, "bounds": bounds,
        "src_idx": np.zeros((n_cores, 128, T), np.int32),
        "m01": np.zeros((n_cores, T, EPT, 64), np.float32),
        "r01": np.zeros((n_cores, T, SPT, EPT), np.float32),
        "scratch_nodes": np.full((n_cores, SPT * T), -1, np.int64),
        "g_row": np.zeros(n_nodes, np.int64),  # node -> global scratch row
        "edge_rows": None,
    }
    for k, (v0, v1, tiles) in enumerate(cores):
        for t, (a, b) in enumerate(tiles):
            nseg = b - a
            rows = np.arange(SPT * t, SPT * t + nseg)
            meta["scratch_nodes"][k, rows] = np.arange(a, b)
            meta["g_row"][a:b] = k * SPT * T + rows
            e0, e1 = seg_start[a], seg_start[b]
            ne = int(e1 - e0)
            assert ne <= EPT
            meta["src_idx"][k, :ne, t] = src_s[e0:e1]
            segl = (dst_s[e0:e1] - a).astype(np.int64)
            m = np.zeros((EPT, SPT), np.float32)
            m[np.arange(ne), segl] = 1.0
            meta["m01"][k, t, :, 16 * (t % 4):16 * (t % 4) + 16] = m
            meta["r01"][k, t] = m.T
    return meta, src_s, dst_s


# ------------------------------------------------------------- device build
def _build(nc, N, T, n_cores=8, phases=4):
    """Emit the full SPMD program. Returns dict of tensor name -> shape info."""
    S = SPT * T           # scratch rows per core
    GS = n_cores * S      # global scratch rows
    NB = (N + 127) // 128  # node blocks for fs projection
    SB = S // 128 * 0 + (S + 127) // 128  # scratch blocks
    assert S % 128 == 0

    # -------- dram tensors
    hT = nc.dram_tensor("hT", [128, N], F32, kind="ExternalInput").ap()
    hTo = nc.dram_tensor("hTo", [128, S], F32, kind="ExternalInput").ap()
    W1s = nc.dram_tensor("W1s", [128, 256], F32, kind="ExternalInput").ap()
    W1d = nc.dram_tensor("W1d", [128, 256], F32, kind="ExternalInput").ap()
    W2s = nc.dram_tensor("W2s", [256, 64], F32, kind="ExternalInput").ap()
    W2d = nc.dram_tensor("W2d", [256, 64], F32, kind="ExternalInput").ap()
    a1r = nc.dram_tensor("a1r", [128, 512], F32, kind="ExternalInput").ap()
    a2r = nc.dram_tensor("a2r", [128, 64], F32, kind="ExternalInput").ap()
    m01 = nc.dram_tensor("m01", [T, EPT, 64], F32, kind="ExternalInput").ap()
    r01 = nc.dram_tensor("r01", [T, SPT, EPT], BF16, kind="ExternalInput").ap()
    sidx = nc.dram_tensor("sidx", [128, T], I32, kind="ExternalInput").ap()
    s2idx = nc.dram_tensor("s2idx", [128, T], I32, kind="ExternalInput").ap()

    fs = nc.dram_tensor("fs", [N, 256], F32, kind="Internal").ap()
    fds_hi = nc.dram_tensor("fds_hi", [S, 256], BF16, kind="Internal").ap()
    fds_lo = nc.dram_tensor("fds_lo", [S, 256], BF16, kind="Internal").ap()
    h1s = nc.dram_tensor("h1s", [S, 256], F32, kind="Internal").ap()
    fd2s_hi = nc.dram_tensor("fd2s_hi", [S, 64], BF16, kind="Internal").ap()
    fd2s_lo = nc.dram_tensor("fd2s_lo", [S, 64], BF16, kind="Internal").ap()
    fs2L = nc.dram_tensor("fs2L", [S, 64], F32, kind="Internal").ap()
    fs2G = nc.dram_tensor("fs2G", [GS, 64], F32, kind="Internal",
                          addr_space="Shared").ap()
    outs = nc.dram_tensor("outs", [S, 64], F32, kind="ExternalOutput").ap()

    with tile.TileContext(nc) as tc:
        # ---- persistent constants
        with tc.tile_pool(name="const", bufs=1) as cp:
            w1s_s = cp.tile([128, 256], F32)
            nc.sync.dma_start(out=w1s_s[:], in_=W1s[:, :])
            w1d_s = cp.tile([128, 256], F32)
            nc.sync.dma_start(out=w1d_s[:], in_=W1d[:, :])
            w2s_s = cp.tile([128, 2, 64], F32)  # [K-block, 2][64]
            nc.sync.dma_start(out=w2s_s[:], in_=W2s[:, :].rearrange("(b p) d -> p b d", p=128))
            w2d_s = cp.tile([128, 2, 64], F32)
            nc.sync.dma_start(out=w2d_s[:], in_=W2d[:, :].rearrange("(b p) d -> p b d", p=128))
            a1_s = cp.tile([128, 512], F32)
            nc.sync.dma_start(out=a1_s[:], in_=a1r[:, :])
            a2_s = cp.tile([128, 64], F32)
            nc.sync.dma_start(out=a2_s[:], in_=a2r[:, :])
            sidx_s = cp.tile([128, T], I32)
            nc.sync.dma_start(out=sidx_s[:], in_=sidx[:, :])
            s2idx_s = cp.tile([128, T], I32)
            nc.sync.dma_start(out=s2idx_s[:], in_=s2idx[:, :])
            ident = cp.tile([128, 128], F32)
            make_identity(nc, ident[:])
            zer = cp.tile([128, 256], F32)
            nc.vector.memset(zer[:], 0.0)

            # ---------------- P0: projections fs (all nodes), fds (own rows)
            with nc.named_scope("P0_proj"), \
                 tc.tile_pool(name="p0ps", bufs=4, space="PSUM") as pp, \
                 tc.tile_pool(name="p0sb", bufs=4) as sb, \
                 tc.tile_pool(name="p0ld", bufs=3) as lp:
                def project(srcT_d, ncols, wtile, dst_d, split=None):
                    # srcT_d: [128, ncols] DRAM (transposed features);
                    # dst_d: [ncols, 256] DRAM = srcT.T @ wtile
                    CH = 1024
                    for c0 in range(0, ncols, CH):
                        cw = min(CH, ncols - c0)
                        ld = lp.tile([128, CH], F32, tag="ld")
                        nc.sync.dma_start(out=ld[:, :cw], in_=srcT_d[:, c0:c0 + cw])
                        for b0 in range(0, cw, 128):
                            nb_ = min(128, cw - b0)
                            ps = pp.tile([128, 256], F32, space="PSUM", tag="ps")
                            nc.tensor.matmul(out=ps[:nb_, :], lhsT=ld[:, b0:b0 + nb_],
                                             rhs=wtile[:], start=True, stop=True)
                            if split is None:
                                st = sb.tile([128, 256], F32, tag="st")
                                nc.vector.tensor_copy(st[:nb_, :], ps[:nb_, :])
                                nc.sync.dma_start(out=dst_d[c0 + b0:c0 + b0 + nb_, :],
                                                  in_=st[:nb_, :])
                            else:
                                hi_d, lo_d = split
                                hi = sb.tile([128, 256], BF16, tag="sthi")
                                nc.vector.tensor_copy(hi[:nb_, :], ps[:nb_, :])
                                lo = sb.tile([128, 256], BF16, tag="stlo")
                                nc.vector.tensor_tensor(out=lo[:nb_, :], in0=ps[:nb_, :],
                                                        in1=hi[:nb_, :], op=AL.subtract)
                                nc.sync.dma_start(out=hi_d[c0 + b0:c0 + b0 + nb_, :],
                                                  in_=hi[:nb_, :])
                                nc.sync.dma_start(out=lo_d[c0 + b0:c0 + b0 + nb_, :],
                                                  in_=lo[:nb_, :])
                project(hT, N, w1s_s, fs)
                project(hTo, S, w1d_s, None, split=(fds_hi, fds_lo))
            if phases >= 1:
              with nc.named_scope("P1_edges1"), \
                 tc.tile_pool(name="p1g", bufs=8) as gp, \
                 tc.tile_pool(name="p1m", bufs=6) as mp, \
                 tc.tile_pool(name="p1w", bufs=4) as wp, \
                 tc.tile_pool(name="p1ps", bufs=4, space="PSUM") as pp, \
                 tc.tile_pool(name="p1fin", bufs=2) as fp:
                for g in range(T // 8):
                    gb = fp.tile([128, 264], F32, tag="gb")
                    m01g = mp.tile([128, 8, 64], F32, tag="m")
                    nc.scalar.dma_start(out=m01g[:], in_=m01[g * 8:(g + 1) * 8, :, :].rearrange("j p c -> p j c"))
                    r01g = mp.tile([SPT, 8, 128], BF16, tag="r")
                    nc.scalar.dma_start(out=r01g[:], in_=r01[g * 8:(g + 1) * 8, :, :].rearrange("j p c -> p j c"))
                    fdg_hi = mp.tile([SPT, 8, 256], BF16, tag="fdh")
                    nc.scalar.dma_start(out=fdg_hi[:], in_=fds_hi[g * 128:(g + 1) * 128, :].rearrange("(j p) d -> p j d", p=SPT))
                    fdg_lo = mp.tile([SPT, 8, 256], BF16, tag="fdl")
                    nc.scalar.dma_start(out=fdg_lo[:], in_=fds_lo[g * 128:(g + 1) * 128, :].rearrange("(j p) d -> p j d", p=SPT))
                    psag = None
                    for jp in range(4):
                        j0 = 2 * jp
                        t0 = g * 8 + j0
                        fst = gp.tile([128, 2, 256], F32, tag="fst")
                        for u in range(2):
                            nc.gpsimd.indirect_dma_start(
                                out=fst[:, u, :], out_offset=None, in_=fs[:, :],
                                in_offset=bass.IndirectOffsetOnAxis(
                                    ap=sidx_s[:, t0 + u:t0 + u + 1], axis=0))
                        psfd = pp.tile([128, 2, 256], F32, space="PSUM", tag="psfd")
                        for u in range(2):
                            nc.tensor.matmul(out=psfd[:, u, :], lhsT=r01g[:, j0 + u, :],
                                             rhs=fdg_hi[:, j0 + u, :], start=True, stop=False)
                            nc.tensor.matmul(out=psfd[:, u, :], lhsT=r01g[:, j0 + u, :],
                                             rhs=fdg_lo[:, j0 + u, :], start=False, stop=True)
                        z = wp.tile([128, 2, 256], F32, tag="z")
                        nc.vector.tensor_tensor(out=z[:], in0=fst[:], in1=psfd[:], op=AL.add)
                        w = wp.tile([128, 2, 256], F32, tag="w")
                        nc.vector.scalar_tensor_tensor(
                            out=w[:], in0=z[:], scalar=NEG_SLOPE, in1=z[:],
                            op0=AL.mult, op1=AL.max)
                        p = wp.tile([128, 2, 8, 32], F32, tag="p")
                        nc.vector.tensor_tensor(
                            out=p[:], in0=w[:].rearrange("e u (h d) -> e u h d", h=8),
                            in1=a1_s[:].rearrange("e (u h d) -> e u h d", u=2, h=8),
                            op=AL.mult)
                        q = gp.tile([128, 2, 264], F32, tag="q")
                        lg = mp.tile([128, 2, 8], F32, tag="lg")
                        nc.vector.tensor_reduce(out=lg[:], in_=p[:],
                                                axis=mybir.AxisListType.X, op=AL.add)
                        nc.scalar.activation(q[:, :, 256:264], lg[:],
                                             mybir.ActivationFunctionType.Exp)
                        nc.vector.tensor_tensor(
                            out=q[:, :, 0:256].rearrange("e u (h d) -> e u h d", h=8),
                            in0=fst[:].rearrange("e u (h d) -> e u h d", h=8),
                            in1=q[:, :, 256:264][:, :, :, None].to_broadcast([128, 2, 8, 32]),
                            op=AL.mult)
                        if j0 % 4 == 0:
                            psag = pp.tile([64, 264], F32, space="PSUM", tag="psag")
                        for u in range(2):
                            j = j0 + u
                            nc.tensor.matmul(out=psag[:], lhsT=m01g[:, j, :], rhs=q[:, u, :],
                                             start=(j % 4 == 0), stop=(j % 4 == 3))
                            if j % 4 == 3:
                                nc.vector.tensor_copy(gb[64 * (j // 4):64 * (j // 4) + 64, :], psag[:])
                    den = mp.tile([128, 8], F32, tag="den")
                    nc.vector.tensor_scalar_max(den[:], gb[:, 256:264], 1e-30)
                    rec = mp.tile([128, 8], F32, tag="rec")
                    nc.vector.reciprocal(rec[:], den[:])
                    o = wp.tile([128, 256], F32, tag="fo")
                    nc.vector.tensor_tensor(
                        out=o[:].rearrange("e (h d) -> e h d", h=8),
                        in0=gb[:, 0:256].rearrange("e (h d) -> e h d", h=8),
                        in1=rec[:][:, :, None].to_broadcast([128, 8, 32]),
                        op=AL.mult)
                    mn = wp.tile([128, 256], F32, tag="fmn")
                    nc.vector.tensor_scalar_min(mn[:], o[:], 0.0)
                    ex = wp.tile([128, 256], F32, tag="fex")
                    nc.scalar.activation(ex[:], mn[:], mybir.ActivationFunctionType.Exp)
                    em1 = wp.tile([128, 256], F32, tag="fem")
                    nc.vector.scalar_tensor_tensor(
                        out=em1[:], in0=ex[:], scalar=1.0, in1=zer[:],
                        op0=AL.subtract, op1=AL.min)
                    mx = wp.tile([128, 256], F32, tag="fmx")
                    nc.vector.tensor_scalar_max(mx[:], o[:], 0.0)
                    h1g = wp.tile([128, 256], F32, tag="fh1")
                    nc.vector.tensor_tensor(out=h1g[:], in0=em1[:], in1=mx[:], op=AL.add)
                    nc.sync.dma_start(out=h1s[g * 128:(g + 1) * 128, :], in_=h1g[:])

            # ---------------- P2: layer-2 projections + AllGather
            if phases >= 2:
              with nc.named_scope("P2_proj2"), \
                 tc.tile_pool(name="p2ps", bufs=4, space="PSUM") as pp, \
                 tc.tile_pool(name="p2sb", bufs=4) as sb:
                for b in range(S // 128):
                    n0 = b * 128
                    blk = sb.tile([128, 256], F32, tag="blk")
                    nc.sync.dma_start(out=blk[:], in_=h1s[n0:n0 + 128, :])
                    h1T = sb.tile([128, 2, 128], F32, tag="h1T")
                    for half in range(2):
                        pst = pp.tile([128, 128], F32, space="PSUM", tag="pst")
                        nc.tensor.transpose(out=pst[:], in_=blk[:, 128 * half:128 * half + 128],
                                            identity=ident[:])
                        nc.vector.tensor_copy(h1T[:, half, :], pst[:])
                    for (wt, dst_t) in ((w2s_s, fs2L), (w2d_s, None)):
                        ps2 = pp.tile([128, 64], F32, space="PSUM", tag="ps2")
                        nc.tensor.matmul(out=ps2[:], lhsT=h1T[:, 0, :], rhs=wt[:, 0, :],
                                         start=True, stop=False)
                        nc.tensor.matmul(out=ps2[:], lhsT=h1T[:, 1, :], rhs=wt[:, 1, :],
                                         start=False, stop=True)
                        if dst_t is not None:
                            st2 = sb.tile([128, 64], F32, tag="st2")
                            nc.vector.tensor_copy(st2[:], ps2[:])
                            nc.sync.dma_start(out=dst_t[n0:n0 + 128, :], in_=st2[:])
                        else:
                            hi2 = sb.tile([128, 64], BF16, tag="hi2")
                            nc.vector.tensor_copy(hi2[:], ps2[:])
                            lo2 = sb.tile([128, 64], BF16, tag="lo2")
                            nc.vector.tensor_tensor(out=lo2[:], in0=ps2[:], in1=hi2[:],
                                                    op=AL.subtract)
                            nc.sync.dma_start(out=fd2s_hi[n0:n0 + 128, :], in_=hi2[:])
                            nc.sync.dma_start(out=fd2s_lo[n0:n0 + 128, :], in_=lo2[:])
                nc.gpsimd.collective_compute(
                    "AllGather", AL.bypass,
                    replica_groups=[list(range(n_cores))],
                    ins=[fs2L[:, :]], outs=[fs2G[:, :]])

            # ---------------- P3: layer-2 edge tiles
            if phases >= 3:
              with nc.named_scope("P3_edges2"), \
                 tc.tile_pool(name="p3g", bufs=8) as gp, \
                 tc.tile_pool(name="p3m", bufs=6) as mp, \
                 tc.tile_pool(name="p3w", bufs=3) as wp, \
                 tc.tile_pool(name="p3ps", bufs=4, space="PSUM") as pp, \
                 tc.tile_pool(name="p3fin", bufs=2) as fp:
                for g in range(T // 8):
                    gb = fp.tile([128, 72], F32, tag="gb2")
                    m01g = mp.tile([128, 8, 64], F32, tag="m")
                    nc.scalar.dma_start(out=m01g[:], in_=m01[g * 8:(g + 1) * 8, :, :].rearrange("j p c -> p j c"))
                    r01g = mp.tile([SPT, 8, 128], BF16, tag="r")
                    nc.scalar.dma_start(out=r01g[:], in_=r01[g * 8:(g + 1) * 8, :, :].rearrange("j p c -> p j c"))
                    fdg_hi = mp.tile([SPT, 8, 64], BF16, tag="fd2h")
                    nc.scalar.dma_start(out=fdg_hi[:], in_=fd2s_hi[g * 128:(g + 1) * 128, :].rearrange("(j p) d -> p j d", p=SPT))
                    fdg_lo = mp.tile([SPT, 8, 64], BF16, tag="fd2l")
                    nc.scalar.dma_start(out=fdg_lo[:], in_=fd2s_lo[g * 128:(g + 1) * 128, :].rearrange("(j p) d -> p j d", p=SPT))
                    psag = None
                    for j in range(8):
                        t = g * 8 + j
                        f2t = gp.tile([128, 64], F32, tag="f2t")
                        nc.gpsimd.indirect_dma_start(
                            out=f2t[:], out_offset=None, in_=fs2G[:, :],
                            in_offset=bass.IndirectOffsetOnAxis(
                                ap=s2idx_s[:, t:t + 1], axis=0))
                        psfd = pp.tile([128, 64], F32, space="PSUM", tag="psfd2")
                        nc.tensor.matmul(out=psfd[:], lhsT=r01g[:, j, :], rhs=fdg_hi[:, j, :],
                                         start=True, stop=False)
                        nc.tensor.matmul(out=psfd[:], lhsT=r01g[:, j, :], rhs=fdg_lo[:, j, :],
                                         start=False, stop=True)
                        z = wp.tile([128, 64], F32, tag="z2l")
                        nc.vector.tensor_tensor(out=z[:], in0=f2t[:], in1=psfd[:], op=AL.add)
                        w = wp.tile([128, 64], F32, tag="w2l")
                        nc.vector.scalar_tensor_tensor(
                            out=w[:], in0=z[:], scalar=NEG_SLOPE, in1=z[:],
                            op0=AL.mult, op1=AL.max)
                        q = gp.tile([128, 72], F32, tag="q2")
                        lg = mp.tile([128, 1], F32, tag="lg2")
                        p2 = wp.tile([128, 1, 64], F32, tag="p2l")
                        nc.vector.tensor_tensor(out=p2[:, 0, :], in0=w[:], in1=a2_s[:],
                                                op=AL.mult)
                        nc.vector.tensor_reduce(out=lg[:], in_=p2[:],
                                                axis=mybir.AxisListType.X, op=AL.add)
                        nc.scalar.activation(q[:, 64:65], lg[:],
                                             mybir.ActivationFunctionType.Exp)
                        nc.vector.tensor_tensor(
                            out=q[:, 0:64], in0=f2t[:],
                            in1=q[:, 64:65].to_broadcast([128, 64]), op=AL.mult)
                        if j % 4 == 0:
                            psag = pp.tile([64, 72], F32, space="PSUM", tag="psag2")
                        nc.tensor.matmul(out=psag[:, 0:65], lhsT=m01g[:, j, :], rhs=q[:, 0:65],
                                         start=(j % 4 == 0), stop=(j % 4 == 3))
                        if j % 4 == 3:
                            nc.vector.tensor_copy(gb[64 * (j // 4):64 * (j // 4) + 64, 0:65],
                                                  psag[:, 0:65])
                    den = mp.tile([128, 1], F32, tag="den2")
                    nc.vector.tensor_scalar_max(den[:], gb[:, 64:65], 1e-30)
                    rec = mp.tile([128, 1], F32, tag="rec2")
                    nc.vector.reciprocal(rec[:], den[:])
                    o = wp.tile([128, 64], F32, tag="o2")
                    nc.vector.tensor_tensor(
                        out=o[:], in0=gb[:, 0:64],
                        in1=rec[:].to_broadcast([128, 64]), op=AL.mult)
                    nc.sync.dma_start(out=outs[g * 128:(g + 1) * 128, :], in_=o[:])

    nc.compile()




def _in_maps(meta, h, W1_src, W1_dst, attn1, W2_src, W2_dst, attn2,
             n_cores=8):
    """Build the per-core input dicts for run_bass_kernel_spmd."""
    T = meta["T"]
    S = SPT * T
    h = np.asarray(h, np.float32)
    a1 = np.asarray(attn1, np.float32).reshape(-1)
    a2 = np.asarray(attn2, np.float32).reshape(-1)
    hT = np.ascontiguousarray(h.T)
    in_maps = []
    for k in range(n_cores):
        sn = meta["scratch_nodes"][k]
        hTo = np.zeros((128, S), np.float32)
        valid = sn >= 0
        hTo[:, valid] = h[sn[valid]].T
        src_idx = meta["src_idx"][k]
        s2 = meta["g_row"][src_idx.astype(np.int64)].astype(np.int32)
        in_maps.append({
            "hT": hT, "hTo": hTo,
            "W1s": np.asarray(W1_src, np.float32),
            "W1d": np.asarray(W1_dst, np.float32),
            "W2s": np.asarray(W2_src, np.float32),
            "W2d": np.asarray(W2_dst, np.float32),
            "a1r": np.ascontiguousarray(np.broadcast_to(np.tile(a1, 2), (128, 512))),
            "a2r": np.ascontiguousarray(np.broadcast_to(a2, (128, 64))),
            "m01": meta["m01"][k],
            "r01": meta["r01"][k].astype(ml_dtypes.bfloat16),
            "sidx": src_idx, "s2idx": s2,
        })
    return in_maps


def _gather_out(res, meta, n_cores=8):
    allrows = np.concatenate([res.results[k]["outs"] for k in range(n_cores)], axis=0)
    return np.ascontiguousarray(allrows[meta["g_row"]].astype(np.float32))


def kernel(h, src, dst, W1_src, W1_dst, attn1, b1, W2_src, W2_dst, attn2, b2):
    h = np.asarray(h, np.float32)
    src = np.asarray(src)
    dst = np.asarray(dst)
    N = h.shape[0]
    assert not np.any(np.asarray(b1)) and not np.any(np.asarray(b2)), \
        "zero biases assumed (spec fill: zeros)"

    n_cores = 8
    meta, _, _ = _prep(src, dst, N, n_cores=n_cores)

    nc = bacc.Bacc("TRN2", target_bir_lowering=False, debug=False,
                   num_devices=n_cores)
    _build(nc, N, meta["T"], n_cores=n_cores)

    in_maps = _in_maps(meta, h, W1_src, W1_dst, attn1, W2_src, W2_dst, attn2,
                       n_cores=n_cores)
    res = run_bass_kernel_spmd(nc, in_maps, core_ids=list(range(n_cores)))
    return _gather_out(res, meta, n_cores=n_cores)



# revision 13
# speedup vs baseline: 1.4690x; 1.4690x over previous
"""Two-layer GATv2 (DGL-style, eval mode) on 8 Trainium2 NeuronCores.

Edge-parallel by destination range: host sorts edges by dst, splits nodes
into 8 contiguous ranges with ~equal edge counts, and packs each range's
dst nodes into tiles of <=128 edges / <=16 segments.

Layer 1 is gather-free: the host pre-slices h.T columns per edge slot
(hsrcT), so z = fs[src]+fd[dst] is two accumulated PE matmuls per tile
(host-sliced source columns @ W1s, plus one-hot segment expansion of the
precomputed dst projection). Logits run Prelu on the scalar engine; q=z*ex
aggregates through a one-hot matmul; out[v] = sum(alpha*z) - fd[v].
Layer 2 projects h1 (fp16), AllGathers fs2, and gathers source rows
per edge tile via indirect DMA. Everything streams fp16; PSUM fp32.
"""
import numpy as np
import ml_dtypes

import concourse.bass as bass
import concourse.tile as tile
from concourse import bacc, mybir
from concourse.bass_utils import run_bass_kernel_spmd
from concourse.masks import make_identity

F32 = mybir.dt.float32
F16 = mybir.dt.float16
I32 = mybir.dt.int32
AL = mybir.AluOpType
AF = mybir.ActivationFunctionType

EPT = 128   # edges per tile
SPT = 16    # segments (dst nodes) per tile
NEG_SLOPE = 0.2


def _prep(src, dst, n_nodes, n_cores=8):
    """Partition + tile the graph. Returns metadata dict."""
    E = src.shape[0]
    src = src.astype(np.int64)
    dst = dst.astype(np.int64)
    order = np.argsort(dst, kind="stable")
    src_s = src[order].astype(np.int32)
    dst_s = dst[order].astype(np.int32)
    deg = np.bincount(dst_s, minlength=n_nodes).astype(np.int64)
    assert deg.max() <= EPT, f"segment larger than a tile: {deg.max()}"
    # node-aligned core boundaries with ~equal edges
    cum = np.cumsum(deg)
    bounds = [0]
    for k in range(1, n_cores):
        t = k * E / n_cores
        bounds.append(int(np.searchsorted(cum, t)))
    bounds.append(n_nodes)
    seg_start = np.concatenate([[0], cum]).astype(np.int64)  # edge offset per node

    cores = []
    for k in range(n_cores):
        v0, v1 = bounds[k], bounds[k + 1]
        tiles = []  # list of (node_lo, node_hi) per tile
        v = v0
        while v < v1:
            ne, ns, vstart = 0, 0, v
            while v < v1 and ns < SPT and ne + deg[v] <= EPT:
                ne += deg[v]; ns += 1; v += 1
            tiles.append((vstart, v))
        cores.append((v0, v1, tiles))
    T = max(len(c[2]) for c in cores)
    T = ((T + 7) // 8) * 8  # multiple of 8 for group finalize

    meta = {
        "T": T, "n_cores": n_cores, "bounds": bounds,
        "src_idx": np.zeros((n_cores, 128, T), np.int32),
        "m01": np.zeros((n_cores, T, EPT, 64), np.float16),
        "r01": np.zeros((n_cores, T, SPT, EPT), np.float16),
        "scratch_nodes": np.full((n_cores, SPT * T), -1, np.int64),
        "g_row": np.zeros(n_nodes, np.int64),  # node -> global scratch row
    }
    for k, (v0, v1, tiles) in enumerate(cores):
        for t, (a, b) in enumerate(tiles):
            nseg = b - a
            rows = np.arange(SPT * t, SPT * t + nseg)
            meta["scratch_nodes"][k, rows] = np.arange(a, b)
            meta["g_row"][a:b] = k * SPT * T + rows
            e0, e1 = seg_start[a], seg_start[b]
            ne = int(e1 - e0)
            assert ne <= EPT
            meta["src_idx"][k, :ne, t] = src_s[e0:e1]
            segl = (dst_s[e0:e1] - a).astype(np.int64)
            m = np.zeros((EPT, SPT), np.float16)
            m[np.arange(ne), segl] = 1.0
            meta["m01"][k, t, :, 16 * (t % 4):16 * (t % 4) + 16] = m
            meta["r01"][k, t] = m.T
    return meta, src_s, dst_s


# ------------------------------------------------------------- device build
def _build(nc, N, T, n_cores=8):
    """Emit the full SPMD program."""
    S = SPT * T           # scratch rows per core
    GS = n_cores * S      # global scratch rows
    assert S % 128 == 0

    # -------- dram tensors
    hsrcT = nc.dram_tensor("hsrcT", [128, T * 128], F16, kind="ExternalInput").ap()
    hTo = nc.dram_tensor("hTo", [128, S], F16, kind="ExternalInput").ap()
    W1s = nc.dram_tensor("W1s", [128, 256], F16, kind="ExternalInput").ap()
    W1d = nc.dram_tensor("W1d", [128, 256], F16, kind="ExternalInput").ap()
    W2s = nc.dram_tensor("W2s", [256, 64], F16, kind="ExternalInput").ap()
    W2d = nc.dram_tensor("W2d", [256, 64], F16, kind="ExternalInput").ap()
    a1r = nc.dram_tensor("a1r", [128, 512], F16, kind="ExternalInput").ap()
    a2r = nc.dram_tensor("a2r", [128, 128], F16, kind="ExternalInput").ap()
    m01 = nc.dram_tensor("m01", [T, EPT, 64], F16, kind="ExternalInput").ap()
    r01 = nc.dram_tensor("r01", [T, SPT, EPT], F16, kind="ExternalInput").ap()
    s2idx = nc.dram_tensor("s2idx", [128, T], I32, kind="ExternalInput").ap()

    fds = nc.dram_tensor("fds", [S, 256], F16, kind="Internal").ap()
    h1s = nc.dram_tensor("h1s", [S, 256], F16, kind="Internal").ap()
    fd2s = nc.dram_tensor("fd2s", [S, 64], F16, kind="Internal").ap()
    fs2L = nc.dram_tensor("fs2L", [S, 64], F16, kind="Internal").ap()
    fs2G = nc.dram_tensor("fs2G", [GS, 64], F16, kind="Internal",
                          addr_space="Shared").ap()
    outs = nc.dram_tensor("outs", [S, 64], F32, kind="ExternalOutput").ap()

    with tile.TileContext(nc) as tc:
        # ---- persistent constants
        with tc.tile_pool(name="const", bufs=1) as cp:
            w1s_s = cp.tile([128, 256], F16)
            nc.sync.dma_start(out=w1s_s[:], in_=W1s[:, :])
            w1d_s = cp.tile([128, 256], F16)
            nc.sync.dma_start(out=w1d_s[:], in_=W1d[:, :])
            w2s_s = cp.tile([128, 2, 64], F16)
            nc.sync.dma_start(out=w2s_s[:], in_=W2s[:, :].rearrange("(b p) d -> p b d", p=128))
            w2d_s = cp.tile([128, 2, 64], F16)
            nc.sync.dma_start(out=w2d_s[:], in_=W2d[:, :].rearrange("(b p) d -> p b d", p=128))
            a1_s = cp.tile([128, 512], F16)
            nc.sync.dma_start(out=a1_s[:], in_=a1r[:, :])
            a2_s = cp.tile([128, 128], F16)
            nc.sync.dma_start(out=a2_s[:], in_=a2r[:, :])
            s2idx_s = cp.tile([128, T], I32)
            nc.sync.dma_start(out=s2idx_s[:], in_=s2idx[:, :])
            ident = cp.tile([128, 128], F16)
            make_identity(nc, ident[:])

            # ---------------- P0: dst projection fds = hTo.T @ W1d (fp16)
            with nc.named_scope("P0_proj"), \
                 tc.tile_pool(name="p0ps", bufs=4, space="PSUM") as pp, \
                 tc.tile_pool(name="p0sb", bufs=4) as sb, \
                 tc.tile_pool(name="p0ld", bufs=3) as lp:
                CH = 1024
                for c0 in range(0, S, CH):
                    cw = min(CH, S - c0)
                    ld = lp.tile([128, CH], F16, tag="ld")
                    nc.sync.dma_start(out=ld[:, :cw], in_=hTo[:, c0:c0 + cw])
                    for b0 in range(0, cw, 128):
                        nb_ = min(128, cw - b0)
                        ps = pp.tile([128, 256], F32, space="PSUM", tag="ps")
                        nc.tensor.matmul(out=ps[:nb_, :], lhsT=ld[:, b0:b0 + nb_],
                                         rhs=w1d_s[:], start=True, stop=True)
                        st = sb.tile([128, 256], F16, tag="st")
                        nc.vector.tensor_copy(st[:nb_, :], ps[:nb_, :])
                        nc.sync.dma_start(out=fds[c0 + b0:c0 + b0 + nb_, :],
                                          in_=st[:nb_, :])

            # ---------------- P1: layer-1 edge tiles (gather-free)
            with nc.named_scope("P1_edges1"), \
                 tc.tile_pool(name="p1g", bufs=6) as gp, \
                 tc.tile_pool(name="p1m", bufs=4) as mp, \
                 tc.tile_pool(name="p1w", bufs=6) as wp, \
                 tc.tile_pool(name="p1ps", bufs=4, space="PSUM") as pp, \
                 tc.tile_pool(name="p1fin", bufs=2) as fp:
                for g in range(T // 8):
                    gb = fp.tile([128, 264], F32, tag="gb")
                    m01g = mp.tile([128, 8, 64], F16, tag="m")
                    nc.scalar.dma_start(out=m01g[:], in_=m01[g * 8:(g + 1) * 8, :, :].rearrange("j p c -> p j c"))
                    r01g = mp.tile([SPT, 8, 128], F16, tag="r")
                    nc.scalar.dma_start(out=r01g[:], in_=r01[g * 8:(g + 1) * 8, :, :].rearrange("j p c -> p j c"))
                    fdg = mp.tile([SPT, 8, 256], F16, tag="fdg")
                    nc.scalar.dma_start(out=fdg[:], in_=fds[g * 128:(g + 1) * 128, :].rearrange("(j p) d -> p j d", p=SPT))
                    hsg = mp.tile([128, 8, 128], F16, tag="hsg")
                    nc.scalar.dma_start(out=hsg[:], in_=hsrcT[:, g * 1024:(g + 1) * 1024].rearrange("p (j e) -> p j e", j=8))
                    psag = None
                    for jp in range(4):
                        j0 = 2 * jp
                        zps = pp.tile([128, 2, 256], F32, space="PSUM", tag="zps")
                        for u in range(2):
                            j = j0 + u
                            nc.tensor.matmul(out=zps[:, u, :], lhsT=hsg[:, j, :],
                                             rhs=w1s_s[:], start=True, stop=False)
                            nc.tensor.matmul(out=zps[:, u, :], lhsT=r01g[:, j, :],
                                             rhs=fdg[:, j, :], start=False, stop=True)
                        w = wp.tile([128, 2, 256], F16, tag="w")
                        nc.scalar.activation(w[:], zps[:], AF.Prelu, alpha=NEG_SLOPE)
                        p = wp.tile([128, 2, 256], F16, tag="p")
                        nc.vector.tensor_tensor(
                            out=p[:], in0=w[:],
                            in1=a1_s[:].rearrange("e (u c) -> e u c", u=2), op=AL.mult)
                        lg = mp.tile([128, 2, 8], F32, tag="lg")
                        nc.vector.tensor_reduce(
                            out=lg[:], in_=p[:].rearrange("e u (h d) -> e u h d", h=8),
                            axis=mybir.AxisListType.X, op=AL.add)
                        q = gp.tile([128, 2, 264], F16, tag="q")
                        exf = mp.tile([128, 2, 8], F32, tag="exf")
                        nc.scalar.activation(exf[:], lg[:], AF.Exp)
                        nc.scalar.activation(q[:, :, 256:264], lg[:], AF.Exp)
                        nc.vector.tensor_tensor(
                            out=q[:, :, 0:256].rearrange("e u (h d) -> e u h d", h=8),
                            in0=zps[:].rearrange("e u (h d) -> e u h d", h=8),
                            in1=exf[:][:, :, :, None].to_broadcast([128, 2, 8, 32]),
                            op=AL.mult)
                        if j0 % 4 == 0:
                            psag = pp.tile([64, 264], F32, space="PSUM", tag="psag")
                        for u in range(2):
                            j = j0 + u
                            nc.tensor.matmul(out=psag[:], lhsT=m01g[:, j, :], rhs=q[:, u, :],
                                             start=(j % 4 == 0), stop=(j % 4 == 3))
                            if j % 4 == 3:
                                nc.vector.tensor_copy(gb[64 * (j // 4):64 * (j // 4) + 64, :], psag[:])
                    den = mp.tile([128, 8], F32, tag="den")
                    nc.vector.tensor_scalar_max(den[:], gb[:, 256:264], 1e-30)
                    rec = mp.tile([128, 8], F32, tag="rec")
                    nc.vector.reciprocal(rec[:], den[:])
                    o = wp.tile([128, 256], F32, tag="fo")
                    nc.vector.tensor_tensor(
                        out=o[:].rearrange("e (h d) -> e h d", h=8),
                        in0=gb[:, 0:256].rearrange("e (h d) -> e h d", h=8),
                        in1=rec[:][:, :, None].to_broadcast([128, 8, 32]),
                        op=AL.mult)
                    fdb = wp.tile([128, 256], F16, tag="fdb")
                    nc.sync.dma_start(out=fdb[:], in_=fds[g * 128:(g + 1) * 128, :])
                    o2 = wp.tile([128, 256], F32, tag="fo2")
                    nc.vector.tensor_tensor(out=o2[:], in0=o[:], in1=fdb[:],
                                            op=AL.subtract)
                    mn = wp.tile([128, 256], F32, tag="fmn")
                    nc.vector.tensor_scalar_min(mn[:], o2[:], 0.0)
                    ex = wp.tile([128, 256], F32, tag="fex")
                    nc.scalar.activation(ex[:], mn[:], AF.Exp)
                    mx = wp.tile([128, 256], F32, tag="fmx")
                    nc.vector.tensor_scalar_max(mx[:], o2[:], 0.0)
                    h1g = wp.tile([128, 256], F16, tag="fh1")
                    nc.vector.scalar_tensor_tensor(
                        out=h1g[:], in0=ex[:], scalar=-1.0, in1=mx[:],
                        op0=AL.add, op1=AL.add)
                    nc.sync.dma_start(out=h1s[g * 128:(g + 1) * 128, :], in_=h1g[:])

            # ---------------- P2: layer-2 projections + AllGather
            with nc.named_scope("P2_proj2"), \
                 tc.tile_pool(name="p2ps", bufs=4, space="PSUM") as pp, \
                 tc.tile_pool(name="p2sb", bufs=4) as sb:
                for b in range(S // 128):
                    n0 = b * 128
                    blk = sb.tile([128, 256], F16, tag="blk")
                    nc.sync.dma_start(out=blk[:], in_=h1s[n0:n0 + 128, :])
                    h1T = sb.tile([128, 2, 128], F16, tag="h1T")
                    for half in range(2):
                        pst = pp.tile([128, 128], F16, space="PSUM", tag="pst")
                        nc.tensor.transpose(out=pst[:], in_=blk[:, 128 * half:128 * half + 128],
                                            identity=ident[:])
                        nc.vector.tensor_copy(h1T[:, half, :], pst[:])
                    for (wt, dst_t) in ((w2s_s, fs2L), (w2d_s, fd2s)):
                        ps2 = pp.tile([128, 64], F32, space="PSUM", tag="ps2")
                        nc.tensor.matmul(out=ps2[:], lhsT=h1T[:, 0, :], rhs=wt[:, 0, :],
                                         start=True, stop=False)
                        nc.tensor.matmul(out=ps2[:], lhsT=h1T[:, 1, :], rhs=wt[:, 1, :],
                                         start=False, stop=True)
                        st2 = sb.tile([128, 64], F16, tag="st2")
                        nc.vector.tensor_copy(st2[:], ps2[:])
                        nc.sync.dma_start(out=dst_t[n0:n0 + 128, :], in_=st2[:])
                nc.gpsimd.collective_compute(
                    "AllGather", AL.bypass,
                    replica_groups=[list(range(n_cores))],
                    ins=[fs2L[:, :]], outs=[fs2G[:, :]])

            # ---------------- P3: layer-2 edge tiles
            with nc.named_scope("P3_edges2"), \
                 tc.tile_pool(name="p3g", bufs=8) as gp, \
                 tc.tile_pool(name="p3m", bufs=6) as mp, \
                 tc.tile_pool(name="p3w", bufs=4) as wp, \
                 tc.tile_pool(name="p3ps", bufs=4, space="PSUM") as pp, \
                 tc.tile_pool(name="p3fin", bufs=2) as fp:
                for g in range(T // 8):
                    gb = fp.tile([128, 72], F32, tag="gb2")
                    m01g = mp.tile([128, 8, 64], F16, tag="m")
                    nc.scalar.dma_start(out=m01g[:], in_=m01[g * 8:(g + 1) * 8, :, :].rearrange("j p c -> p j c"))
                    r01g = mp.tile([SPT, 8, 128], F16, tag="r")
                    nc.scalar.dma_start(out=r01g[:], in_=r01[g * 8:(g + 1) * 8, :, :].rearrange("j p c -> p j c"))
                    fd2g = mp.tile([SPT, 8, 64], F16, tag="fd2g")
                    nc.scalar.dma_start(out=fd2g[:], in_=fd2s[g * 128:(g + 1) * 128, :].rearrange("(j p) d -> p j d", p=SPT))
                    psag = None
                    for jp in range(4):
                        j0 = 2 * jp
                        f2t = gp.tile([128, 2, 64], F16, tag="f2t")
                        psfd = pp.tile([128, 2, 64], F32, space="PSUM", tag="psfd2")
                        for u in range(2):
                            j = j0 + u
                            nc.gpsimd.indirect_dma_start(
                                out=f2t[:, u, :], out_offset=None, in_=fs2G[:, :],
                                in_offset=bass.IndirectOffsetOnAxis(
                                    ap=s2idx_s[:, g * 8 + j:g * 8 + j + 1], axis=0))
                            nc.tensor.matmul(out=psfd[:, u, :], lhsT=r01g[:, j, :],
                                             rhs=fd2g[:, j, :], start=True, stop=True)
                        sd = wp.tile([128, 2, 64], F16, tag="sd")
                        nc.scalar.activation(sd[:], psfd[:], AF.Copy)
                        z = wp.tile([128, 2, 64], F16, tag="z2l")
                        nc.vector.tensor_tensor(out=z[:], in0=f2t[:], in1=sd[:], op=AL.add)
                        w = wp.tile([128, 2, 64], F16, tag="w2l")
                        nc.scalar.activation(w[:], z[:], AF.Prelu, alpha=NEG_SLOPE)
                        p2 = wp.tile([128, 2, 64], F16, tag="p2l")
                        nc.vector.tensor_tensor(
                            out=p2[:], in0=w[:],
                            in1=a2_s[:].rearrange("e (u c) -> e u c", u=2), op=AL.mult)
                        lg = mp.tile([128, 2, 1], F32, tag="lg2")
                        nc.vector.tensor_reduce(out=lg[:], in_=p2[:],
                                                axis=mybir.AxisListType.X, op=AL.add)
                        q = gp.tile([128, 2, 72], F16, tag="q2")
                        exf = mp.tile([128, 2, 1], F32, tag="exf2")
                        nc.scalar.activation(exf[:], lg[:], AF.Exp)
                        nc.scalar.activation(q[:, :, 64:65], lg[:], AF.Exp)
                        nc.vector.tensor_tensor(
                            out=q[:, :, 0:64], in0=f2t[:],
                            in1=exf[:].to_broadcast([128, 2, 64]), op=AL.mult)
                        if j0 % 4 == 0:
                            psag = pp.tile([64, 72], F32, space="PSUM", tag="psag2")
                        for u in range(2):
                            j = j0 + u
                            nc.tensor.matmul(out=psag[:, 0:65], lhsT=m01g[:, j, :],
                                             rhs=q[:, u, 0:65],
                                             start=(j % 4 == 0), stop=(j % 4 == 3))
                            if j % 4 == 3:
                                nc.vector.tensor_copy(gb[64 * (j // 4):64 * (j // 4) + 64, 0:65],
                                                      psag[:, 0:65])
                    den = mp.tile([128, 1], F32, tag="den2")
                    nc.vector.tensor_scalar_max(den[:], gb[:, 64:65], 1e-30)
                    rec = mp.tile([128, 1], F32, tag="rec2")
                    nc.vector.reciprocal(rec[:], den[:])
                    o = wp.tile([128, 64], F32, tag="o2")
                    nc.vector.tensor_tensor(
                        out=o[:], in0=gb[:, 0:64],
                        in1=rec[:].to_broadcast([128, 64]), op=AL.mult)
                    nc.sync.dma_start(out=outs[g * 128:(g + 1) * 128, :], in_=o[:])

    nc.compile()


def _in_maps(meta, h, W1_src, W1_dst, attn1, W2_src, W2_dst, attn2,
             n_cores=8):
    """Build the per-core input dicts for run_bass_kernel_spmd."""
    T = meta["T"]
    S = SPT * T
    h = np.asarray(h, np.float32)
    a1 = np.asarray(attn1, np.float32).reshape(-1)
    a2 = np.asarray(attn2, np.float32).reshape(-1)
    hT16 = np.ascontiguousarray(h.T.astype(np.float16))
    in_maps = []
    for k in range(n_cores):
        sn = meta["scratch_nodes"][k]
        hTo = np.zeros((128, S), np.float16)
        valid = sn >= 0
        hTo[:, valid] = hT16[:, sn[valid]]
        src_idx = meta["src_idx"][k]          # [128, T] slot -> src node
        # host-sliced source columns: [128, T*128], slot-major per tile
        hsrcT = np.ascontiguousarray(
            hT16[:, src_idx.T.reshape(-1)])   # cols ordered (t, e)
        s2 = meta["g_row"][src_idx.astype(np.int64)].astype(np.int32)
        in_maps.append({
            "hsrcT": hsrcT, "hTo": hTo,
            "W1s": np.asarray(W1_src, np.float16),
            "W1d": np.asarray(W1_dst, np.float16),
            "W2s": np.asarray(W2_src, np.float16),
            "W2d": np.asarray(W2_dst, np.float16),
            "a1r": np.ascontiguousarray(np.broadcast_to(
                np.tile(a1, 2), (128, 512)).astype(np.float16)),
            "a2r": np.ascontiguousarray(np.broadcast_to(
                np.tile(a2, 2), (128, 128)).astype(np.float16)),
            "m01": meta["m01"][k],
            "r01": meta["r01"][k],
            "s2idx": s2,
        })
    return in_maps


def _gather_out(res, meta, n_cores=8):
    allrows = np.concatenate([res.results[k]["outs"] for k in range(n_cores)], axis=0)
    return np.ascontiguousarray(allrows[meta["g_row"]].astype(np.float32))


def kernel(h, src, dst, W1_src, W1_dst, attn1, b1, W2_src, W2_dst, attn2, b2):
    h = np.asarray(h, np.float32)
    src = np.asarray(src)
    dst = np.asarray(dst)
    N = h.shape[0]
    assert not np.any(np.asarray(b1)) and not np.any(np.asarray(b2)), \
        "zero biases assumed (spec fill: zeros)"

    n_cores = 8
    meta, _, _ = _prep(src, dst, N, n_cores=n_cores)

    nc = bacc.Bacc("TRN2", target_bir_lowering=False, debug=False,
                   num_devices=n_cores)
    _build(nc, N, meta["T"], n_cores=n_cores)

    in_maps = _in_maps(meta, h, W1_src, W1_dst, attn1, W2_src, W2_dst, attn2,
                       n_cores=n_cores)
    res = run_bass_kernel_spmd(nc, in_maps, core_ids=list(range(n_cores)))
    return _gather_out(res, meta, n_cores=n_cores)
